# revision 7
# baseline (speedup 1.0000x reference)
"""Trainium2 Bass kernel for the reversible-GRU RNN language model.

Pipeline (per 16-step chunk, 8 chunks):
  gather(embed rows) -> PE-transpose to feature-major -> L1 x-projection GEMM
  -> L1 recurrent scan -> L2 x-projection GEMM -> L2 scan -> vocab-sharded
  tied-decoder GEMM (+bias) -> logits DMA out.

Sharding: gather/scan replicated on all 8 cores (latency-bound), decoder GEMM
sharded on the vocab axis (each core computes 4160 of 33280 padded columns).
All intermediates stay in SBUF; the recurrence state lives in rolling 16-step
stage buffers that double as the next phase's GEMM inputs.
"""

import sys
from contextlib import ExitStack

import numpy as np

sys.path.insert(0, "/opt/trn_rl_repo")

import ml_dtypes  # noqa: E402
import concourse.bass as bass  # noqa: E402
import concourse.tile as tile  # noqa: E402
import concourse.mybir as mybir  # noqa: E402
from concourse import bacc  # noqa: E402
from concourse.bass_utils import run_bass_kernel_spmd  # noqa: E402
from concourse.masks import make_identity  # noqa: E402

L, S, B, NTOK, D = 2, 128, 32, 33278, 512
H2 = D // 2
MF = 0.875
P = 128
NT = S * B              # 4096 tokens
NCORES = 8
VP = 33280              # padded vocab
VS = VP // NCORES       # 4160 per-core vocab shard
NCH = 16                # chunks
CS = S // NCH           # 8 steps per chunk
TOKC = CS * B           # 256 tokens per chunk
GPC = TOKC // P         # gather/decode token-tiles per chunk
GF = 1536               # total gate features: zr1(512) g1(256) zr2(512) g2(256)
NM = GF // P            # 12 gate-feature tiles
# gate feature tile offsets (in units of 128-feature tiles)
M_ZR1, M_G1, M_ZR2, M_G2 = 0, 4, 6, 10

F32 = mybir.dt.float32
BF16 = mybir.dt.bfloat16
I32 = mybir.dt.int32
AF = mybir.ActivationFunctionType
ALU = mybir.AluOpType

# ---------------------------------------------------------------------------
# Device program
# ---------------------------------------------------------------------------


def build_program(preload_psum: bool = False):
    nc = bacc.Bacc("TRN2", target_bir_lowering=False, debug=False,
                   num_devices=NCORES)

    # -------- DRAM I/O --------
    idx_d = nc.dram_tensor("idx", [NT, 1], I32, kind="ExternalInput").ap()
    h0_d = nc.dram_tensor("h0", [L, B, D], F32, kind="ExternalInput").ap()
    embed_d = nc.dram_tensor("embed", [NTOK, D], F32, kind="ExternalInput").ap()
    embt_d = nc.dram_tensor("embt", [D, VS], BF16, kind="ExternalInput").ap()
    vbias_d = nc.dram_tensor("vbias", [P, VS], BF16, kind="ExternalInput").ap()
    wx_d = nc.dram_tensor("wx", [L, D, GF], BF16, kind="ExternalInput").ap()
    wh_d = nc.dram_tensor("wh", [L, H2, GF], F32, kind="ExternalInput").ap()
    ball_d = nc.dram_tensor("ball", [L, NM, P], F32, kind="ExternalInput").ap()

    logits_d = nc.dram_tensor("logits", [NT, VS], F32, kind="ExternalOutput").ap()
    hlast_d = nc.dram_tensor("hlast", [L, B, D], F32, kind="ExternalOutput").ap()

    with tile.TileContext(nc) as tc, ExitStack() as ctx:
        const = ctx.enter_context(tc.tile_pool(name="const", bufs=1))
        gpool = ctx.enter_context(tc.tile_pool(name="gather", bufs=3))
        ipool = ctx.enter_context(tc.tile_pool(name="idx", bufs=4))
        xpool = ctx.enter_context(tc.tile_pool(name="xst", bufs=2))
        ppool = ctx.enter_context(tc.tile_pool(name="pst", bufs=2))
        ypool = ctx.enter_context(tc.tile_pool(name="yst", bufs=2))
        wpool = ctx.enter_context(tc.tile_pool(name="work", bufs=2))
        dpool = ctx.enter_context(tc.tile_pool(name="dec", bufs=3))
        opool = ctx.enter_context(tc.tile_pool(name="dout", bufs=4))
        # PSUM pools: total tag-slots must fit in 8 banks.
        ps_big = ctx.enter_context(tc.tile_pool(name="psbig", bufs=3, space="PSUM"))
        ps_tp = ctx.enter_context(tc.tile_pool(name="pstp", bufs=2, space="PSUM"))
        ps_scan = ctx.enter_context(tc.tile_pool(name="psscan", bufs=3, space="PSUM"))

        # -------- constants / weights --------
        ident = const.tile([P, P], F32, tag="ident")
        make_identity(nc, ident[:])

        wx_sb = []
        wh_sb = []
        for l in range(L):
            t = const.tile([P, 4 * GF], BF16, tag=f"wx{l}")
            nc.sync.dma_start(t[:].rearrange("p (k m) -> p k m", k=4),
                              wx_d[l].rearrange("(k p) m -> p k m", p=P))
            wx_sb.append(t)
            t = const.tile([P, 2 * GF], F32, tag=f"wh{l}")
            nc.sync.dma_start(t[:].rearrange("p (k m) -> p k m", k=2),
                              wh_d[l].rearrange("(k p) m -> p k m", p=P))
            wh_sb.append(t)

        ball_sb = const.tile([P, L * NM], F32, tag="ball")
        nc.sync.dma_start(ball_sb[:].rearrange("p (l m) -> p l m", l=L),
                          ball_d.rearrange("l m p -> p l m"))

        embt_sb = const.tile([P, 4 * VS], BF16, tag="embt")
        nc.sync.dma_start(embt_sb[:].rearrange("p (k v) -> p k v", k=4),
                          embt_d.rearrange("(k p) v -> p k v", p=P))

        vbias_sb = const.tile([P, VS], BF16, tag="vbias")
        nc.sync.dma_start(vbias_sb[:], vbias_d[:, :])

        # initial hidden state, transposed to feature-major [P, 4, B]
        h0t = []
        for l in range(L):
            hsb = wpool.tile([B, D], F32, tag="h0load")
            nc.sync.dma_start(hsb[:], h0_d[l])
            ht = const.tile([P, 4 * B], F32, tag=f"h0t{l}")
            for kk in range(4):
                tp = ps_tp.tile([P, P], F32, tag="tp")
                nc.tensor.transpose(tp[:, :B], hsb[:, kk * P:(kk + 1) * P],
                                    ident[:B, :B])
                nc.vector.tensor_copy(ht[:, kk * B:(kk + 1) * B], tp[:, :B])
            h0t.append(ht)

        # per-layer lhsT slice helpers
        def wx_lhsT(l, k, m):
            return wx_sb[l][:, k * GF + m * P: k * GF + (m + 1) * P]

        def wh_lhsT(l, k, m):
            return wh_sb[l][:, k * GF + m * P: k * GF + (m + 1) * P]

        # rolling stage state (python refs to previous chunk's tiles)
        prev_y = [None, None]     # previous chunk ystage per layer
        xst_cur = [None]          # current chunk xstage
        p_cur = [None, None]      # current chunk gate-preact stage per layer
        y_cur = [None, None]      # current chunk ystage per layer

        # ------------------------------------------------------------------
        def emit_gather(c):
            xst = xpool.tile([P, 4, TOKC], BF16, tag="xst")
            for g in range(GPC):
                gt = c * GPC + g
                it = ipool.tile([P, 1], I32, tag="idx")
                nc.sync.dma_start(it[:], idx_d[gt * P:(gt + 1) * P, :])
                xg = gpool.tile([P, D], F32, tag="xg")
                nc.gpsimd.indirect_dma_start(
                    out=xg[:], out_offset=None, in_=embed_d[:, :],
                    in_offset=bass.IndirectOffsetOnAxis(ap=it[:, :1], axis=0))
                for k in range(4):
                    tp = ps_tp.tile([P, P], F32, tag="tp")
                    nc.tensor.transpose(tp[:], xg[:, k * P:(k + 1) * P], ident[:])
                    nc.scalar.copy(xst[:, k, g * P:(g + 1) * P], tp[:])
            xst_cur[0] = xst

        # ------------------------------------------------------------------
        def emit_precompute(l, c):
            """x-projection GEMM for chunk c of layer l -> pstage (bf16)."""
            if l == 0:
                rhs = [xst_cur[0][:, k, :] for k in range(4)]
            else:
                rhs = []
                for k in range(4):
                    cast = wpool.tile([P, TOKC], BF16, tag=f"ycast{k}")
                    nc.vector.tensor_copy(cast[:], prev_or_cur_y(l - 1)[:, k, :])
                    rhs.append(cast[:])
            pst = ppool.tile([P, NM, TOKC], BF16, tag=f"pst{l}")
            for m in range(NM):
                ps = ps_big.tile([P, 512], F32, tag="big")
                for k in range(4):
                    nc.tensor.matmul(ps[:, :TOKC], wx_lhsT(l, k, m), rhs[k],
                                     start=(k == 0), stop=(k == 3))
                nc.scalar.activation(pst[:, m, :], ps[:, :TOKC], AF.Identity,
                                     bias=ball_sb[:, l * NM + m: l * NM + m + 1],
                                     scale=1.0)
            p_cur[l] = pst

        def prev_or_cur_y(l):
            return y_cur[l][:]

        # ------------------------------------------------------------------
        def emit_scan_chunk(l, c):
            """16 recurrent steps for layer l, chunk c."""
            yst = ypool.tile([P, 4, TOKC], F32, tag=f"ys{l}")
            pst = p_cur[l]
            for tl in range(CS):
                t = c * CS + tl
                # h(t-1) access
                if t == 0:
                    hb = h0t[l][:].rearrange("p (k b) -> p k b", k=4)
                    pc = 0
                elif tl == 0:
                    hb = prev_y[l][:]
                    pc = (CS - 1) * B
                else:
                    hb = yst[:]
                    pc = (tl - 1) * B
                tcol = tl * B

                h1p = hb[:, 0:2, pc:pc + B]   # [P, 2, B] old h1
                h2p = hb[:, 2:4, pc:pc + B]   # [P, 2, B] old h2

                # ---- half 1: update h1 using h2p ----
                emit_half(l, pst, yst, tcol,
                          h_in=h2p, h_self=h1p,
                          m_zr=M_ZR1, m_g=M_G1, out_k=0)
                h1n = yst[:, 0:2, tcol:tcol + B]
                # ---- half 2: update h2 using h1n ----
                emit_half(l, pst, yst, tcol,
                          h_in=h1n, h_self=h2p,
                          m_zr=M_ZR2, m_g=M_G2, out_k=2)
            prev_y[l] = yst
            y_cur[l] = yst

        def emit_half(l, pst, yst, tcol, h_in, h_self, m_zr, m_g, out_k):
            """One reversible-GRU half-update.

            u_zr = P_zr + h_in @ Wzr_h.T ; s = sigmoid(u_zr); z*, r = s
            u_g = P_g + (r*h_in) @ Wg_h.T ; g = tanh(u_g)
            h_new = (0.875 z* + 0.125) h_self + 0.875 (1 - z*) g
            """
            ps_zr = ps_scan.tile([P, 192], F32, tag="sps")
            for m in range(4):
                for k in range(2):
                    nc.tensor.matmul(
                        ps_zr[:, m * B:(m + 1) * B],
                        wh_lhsT(l, k, m_zr + m), h_in[:, k, :],
                        start=(k == 0), stop=(k == 1))
            u_zr = wpool.tile([P, 4, B], F32, tag="uzr")
            nc.vector.scalar_tensor_tensor(
                out=u_zr[:], in0=ps_zr[:, 0:128].rearrange("p (m b) -> p m b", m=4),
                scalar=0.0, in1=pst[:, m_zr:m_zr + 4, tcol:tcol + B],
                op0=ALU.add, op1=ALU.add)
            sig = wpool.tile([P, 4, B], F32, tag="sig")
            nc.scalar.activation(sig[:], u_zr[:], AF.Sigmoid)
            zs = sig[:, 0:2, :]               # [P, 2, B]
            rs = sig[:, 2:4, :]
            # r * h_in
            rh = wpool.tile([P, 2, B], F32, tag="rh")
            nc.vector.tensor_tensor(out=rh[:], in0=rs, in1=h_in, op=ALU.mult)
            # g matmuls
            for m in range(2):
                for k in range(2):
                    nc.tensor.matmul(
                        ps_zr[:, 128 + m * B: 128 + (m + 1) * B],
                        wh_lhsT(l, k, m_g + m), rh[:, k, :],
                        start=(k == 0), stop=(k == 1))
            u_g = wpool.tile([P, 2, B], F32, tag="ug")
            nc.vector.scalar_tensor_tensor(
                out=u_g[:], in0=ps_zr[:, 128:192].rearrange("p (m b) -> p m b", m=2),
                scalar=0.0, in1=pst[:, m_g:m_g + 2, tcol:tcol + B],
                op0=ALU.add, op1=ALU.add)
            g = wpool.tile([P, 2, B], F32, tag="g")
            nc.scalar.activation(g[:], u_g[:], AF.Tanh)
            # off-critical-path: zp = MF*z + (1-MF); w = MF*(1-z); a = zp*h_self
            zp = wpool.tile([P, 2, B], F32, tag="zp")
            nc.vector.tensor_scalar(zp[:], zs, MF, 1.0 - MF, ALU.mult, ALU.add)
            w = wpool.tile([P, 2, B], F32, tag="w")
            nc.vector.tensor_scalar(w[:], zs, -MF, MF, ALU.mult, ALU.add)
            a = wpool.tile([P, 2, B], F32, tag="a")
            nc.vector.tensor_tensor(out=a[:], in0=zp[:], in1=h_self, op=ALU.mult)
            # critical: cgw = g*w ; h_new = cgw + a
            cgw = wpool.tile([P, 2, B], F32, tag="cgw")
            nc.vector.tensor_tensor(out=cgw[:], in0=g[:], in1=w[:], op=ALU.mult)
            nc.vector.tensor_tensor(out=yst[:, out_k:out_k + 2, tcol:tcol + B],
                                    in0=cgw[:], in1=a[:], op=ALU.add)

        # ------------------------------------------------------------------
        def emit_decode(c):
            """Tied-decoder GEMM for the 4 token-tiles of chunk c."""
            yst = y_cur[1]
            for mi in range(GPC):
                tok0 = mi * P
                lhs = []
                for k in range(4):
                    cast = dpool.tile([P, P], BF16, tag=f"dlhs{k}")
                    nc.vector.tensor_copy(cast[:], yst[:, k, tok0:tok0 + P])
                    lhs.append(cast)
                mt = c * GPC + mi
                for n in range(9):
                    nsz = 512 if n < 8 else VS - 8 * 512
                    ps = ps_big.tile([P, 512], F32, tag="big")
                    for k in range(4):
                        nc.tensor.matmul(
                            ps[:, :nsz], lhs[k][:],
                            embt_sb[:, k * VS + n * 512: k * VS + n * 512 + nsz],
                            start=(k == 0), stop=(k == 3))
                    ot = opool.tile([P, 512], F32, tag="ot")
                    nc.vector.scalar_tensor_tensor(
                        out=ot[:, :nsz], in0=ps[:, :nsz], scalar=0.0,
                        in1=vbias_sb[:, n * 512: n * 512 + nsz],
                        op0=ALU.add, op1=ALU.add)
                    nc.sync.dma_start(
                        logits_d[mt * P:(mt + 1) * P, n * 512: n * 512 + nsz],
                        ot[:, :nsz])

        # ------------------------------------------------------------------
        def emit_hlast(l):
            yst = y_cur[l]
            hp = ps_tp.tile([B, 4 * P], F32, tag="tp")
            for kk in range(4):
                nc.tensor.transpose(hp[:, kk * P:(kk + 1) * P],
                                    yst[:, kk, (CS - 1) * B: CS * B],
                                    ident[:])
            hs = wpool.tile([B, D], F32, tag="hlast")
            nc.vector.tensor_copy(hs[:], hp[:])
            nc.sync.dma_start(hlast_d[l], hs[:])

        # ------------------------------------------------------------------
        # pipeline emission
        for c in range(NCH):
            emit_gather(c)
            emit_precompute(0, c)
            emit_scan_chunk(0, c)
            emit_precompute(1, c)
            emit_scan_chunk(1, c)
            emit_decode(c)
        emit_hlast(0)
        emit_hlast(1)

    nc.compile()
    return nc


# ---------------------------------------------------------------------------
# Host side
# ---------------------------------------------------------------------------

_prog_cache = {}


def _get_program():
    if "nc" not in _prog_cache:
        _prog_cache["nc"] = build_program()
    return _prog_cache["nc"]


def _prep_inputs(input_seq, hiddens, embed, out_bias,
                 W_zr1, b_zr1, W_g1, b_g1, W_zr2, b_zr2, W_g2, b_g2):
    idx = np.asarray(input_seq).astype(np.int32).reshape(NT, 1)
    h0 = np.asarray(hiddens, dtype=np.float32)
    embed = np.ascontiguousarray(np.asarray(embed, dtype=np.float32))

    # gate weight packing: x-part lhsT [L, D, GF] (bf16), h-part lhsT [L, H2, GF]
    W_zr1 = np.asarray(W_zr1, np.float32)
    W_g1 = np.asarray(W_g1, np.float32)
    W_zr2 = np.asarray(W_zr2, np.float32)
    W_g2 = np.asarray(W_g2, np.float32)
    wx = np.concatenate([W_zr1[:, :, :D], W_g1[:, :, :D],
                         W_zr2[:, :, :D], W_g2[:, :, :D]], axis=1)  # [L, GF, D]
    wx = np.ascontiguousarray(wx.transpose(0, 2, 1)).astype(ml_dtypes.bfloat16)
    whp = np.concatenate([W_zr1[:, :, D:], W_g1[:, :, D:],
                          W_zr2[:, :, D:], W_g2[:, :, D:]], axis=1)  # [L, GF, H2]
    whp = np.ascontiguousarray(whp.transpose(0, 2, 1)).astype(np.float32)
    ball = np.concatenate([np.asarray(b_zr1, np.float32),
                           np.asarray(b_g1, np.float32),
                           np.asarray(b_zr2, np.float32),
                           np.asarray(b_g2, np.float32)], axis=1)  # [L, GF]
    ball = np.ascontiguousarray(ball.reshape(L, NM, P))

    ob = np.zeros(VP, np.float32)
    ob[:NTOK] = np.asarray(out_bias, np.float32)

    embed_pad = np.zeros((VP, D), np.float32)
    embed_pad[:NTOK] = embed

    in_maps = []
    for c in range(NCORES):
        sl = slice(c * VS, (c + 1) * VS)
        embt = np.ascontiguousarray(embed_pad[sl].T).astype(ml_dtypes.bfloat16)
        vbias = np.ascontiguousarray(
            np.broadcast_to(ob[sl][None, :], (P, VS))).astype(ml_dtypes.bfloat16)
        in_maps.append({
            "idx": idx, "h0": h0, "embed": embed,
            "embt": embt, "vbias": vbias,
            "wx": wx, "wh": whp, "ball": ball,
        })
    return in_maps


def run(inputs, trace=False, tmpdir=None):
    nc = _get_program()
    in_maps = _prep_inputs(**inputs)
    res = run_bass_kernel_spmd(nc, in_maps, list(range(NCORES)), trace=trace,
                               tmpdir=tmpdir)
    logits = np.concatenate([res.results[c]["logits"] for c in range(NCORES)],
                            axis=1)[:, :NTOK]
    decoded = logits.reshape(S, B, NTOK)
    hlast = res.results[0]["hlast"]
    return (decoded, hlast), res


def kernel(**inputs):
    out, _ = run(inputs)
    return out


# revision 9
# speedup vs baseline: 2.1978x; 2.1978x over previous
"""Trainium2 Bass kernel for the reversible-GRU RNN language model.

Pipeline (per 16-step chunk, 8 chunks):
  gather(embed rows) -> PE-transpose to feature-major -> L1 x-projection GEMM
  -> L1 recurrent scan -> L2 x-projection GEMM -> L2 scan -> vocab-sharded
  tied-decoder GEMM (+bias) -> logits DMA out.

Sharding: gather/scan replicated on all 8 cores (latency-bound), decoder GEMM
sharded on the vocab axis (each core computes 4160 of 33280 padded columns).
All intermediates stay in SBUF; the recurrence state lives in rolling 16-step
stage buffers that double as the next phase's GEMM inputs.
"""

import sys
from contextlib import ExitStack

import numpy as np

sys.path.insert(0, "/opt/trn_rl_repo")

import ml_dtypes  # noqa: E402
import concourse.bass as bass  # noqa: E402
import concourse.tile as tile  # noqa: E402
import concourse.mybir as mybir  # noqa: E402
from concourse import bacc  # noqa: E402
from concourse.bass_utils import run_bass_kernel_spmd  # noqa: E402
from concourse.masks import make_identity  # noqa: E402

L, S, B, NTOK, D = 2, 128, 32, 33278, 512
H2 = D // 2
MF = 0.875
P = 128
NT = S * B              # 4096 tokens
NCORES = 8
VP = 33280              # padded vocab
VS = VP // NCORES       # 4160 per-core vocab shard
NCH = 16                # chunks
CS = S // NCH           # 8 steps per chunk
TOKC = CS * B           # 256 tokens per chunk
GPC = TOKC // P         # gather/decode token-tiles per chunk
GF = 1536               # total gate features: zr1(512) g1(256) zr2(512) g2(256)
NM = GF // P            # 12 gate-feature tiles
# gate feature tile offsets (in units of 128-feature tiles)
M_ZR1, M_G1, M_ZR2, M_G2 = 0, 4, 6, 10

F32 = mybir.dt.float32
BF16 = mybir.dt.bfloat16
I32 = mybir.dt.int32
AF = mybir.ActivationFunctionType
ALU = mybir.AluOpType

# ---------------------------------------------------------------------------
# Device program
# ---------------------------------------------------------------------------


def build_program(preload_psum: bool = False):
    nc = bacc.Bacc("TRN2", target_bir_lowering=False, debug=False,
                   num_devices=NCORES)

    # -------- DRAM I/O --------
    idx_d = nc.dram_tensor("idx", [NT, 1], I32, kind="ExternalInput").ap()
    h0_d = nc.dram_tensor("h0", [L, B, D], F32, kind="ExternalInput").ap()
    embed_d = nc.dram_tensor("embed", [NTOK, D], F32, kind="ExternalInput").ap()
    embt_d = nc.dram_tensor("embt", [D, VS], BF16, kind="ExternalInput").ap()
    vbias_d = nc.dram_tensor("vbias", [P, VS], BF16, kind="ExternalInput").ap()
    wx_d = nc.dram_tensor("wx", [L, D, GF], BF16, kind="ExternalInput").ap()
    wh_d = nc.dram_tensor("wh", [L, H2, GF], BF16, kind="ExternalInput").ap()
    ball_d = nc.dram_tensor("ball", [L, NM, P], F32, kind="ExternalInput").ap()

    logits_d = nc.dram_tensor("logits", [NT, VS], F32, kind="ExternalOutput").ap()
    hlast_d = nc.dram_tensor("hlast", [L, B, D], F32, kind="ExternalOutput").ap()

    with tile.TileContext(nc) as tc, ExitStack() as ctx:
        const = ctx.enter_context(tc.tile_pool(name="const", bufs=1))
        gpool = ctx.enter_context(tc.tile_pool(name="gather", bufs=3))
        ipool = ctx.enter_context(tc.tile_pool(name="idx", bufs=4))
        xpool = ctx.enter_context(tc.tile_pool(name="xst", bufs=2))
        ppool = ctx.enter_context(tc.tile_pool(name="pst", bufs=2))
        ypool = ctx.enter_context(tc.tile_pool(name="yst", bufs=2))
        wpool = ctx.enter_context(tc.tile_pool(name="work", bufs=2))
        dpool = ctx.enter_context(tc.tile_pool(name="dec", bufs=3))
        opool = ctx.enter_context(tc.tile_pool(name="dout", bufs=4))
        # PSUM pools: total tag-slots must fit in 8 banks.
        ps_big = ctx.enter_context(tc.tile_pool(name="psbig", bufs=3, space="PSUM"))
        ps_tp = ctx.enter_context(tc.tile_pool(name="pstp", bufs=2, space="PSUM"))
        ps_scan = ctx.enter_context(tc.tile_pool(name="psscan", bufs=3, space="PSUM"))

        # -------- constants / weights --------
        ident = const.tile([P, P], F32, tag="ident")
        make_identity(nc, ident[:])
        identb = const.tile([P, P], BF16, tag="identb")
        nc.vector.tensor_copy(identb[:], ident[:])

        wx_sb = []
        wh_sb = []
        for l in range(L):
            t = const.tile([P, 4 * GF], BF16, tag=f"wx{l}")
            nc.sync.dma_start(t[:].rearrange("p (k m) -> p k m", k=4),
                              wx_d[l].rearrange("(k p) m -> p k m", p=P))
            wx_sb.append(t)
            t = const.tile([P, 2 * GF], BF16, tag=f"wh{l}")
            nc.sync.dma_start(t[:].rearrange("p (k m) -> p k m", k=2),
                              wh_d[l].rearrange("(k p) m -> p k m", p=P))
            wh_sb.append(t)

        ball_sb = const.tile([P, L * NM], F32, tag="ball")
        nc.sync.dma_start(ball_sb[:].rearrange("p (l m) -> p l m", l=L),
                          ball_d.rearrange("l m p -> p l m"))

        embt_sb = const.tile([P, 4 * VS], BF16, tag="embt")
        nc.sync.dma_start(embt_sb[:].rearrange("p (k v) -> p k v", k=4),
                          embt_d.rearrange("(k p) v -> p k v", p=P))

        vbias_sb = const.tile([P, VS], BF16, tag="vbias")
        nc.sync.dma_start(vbias_sb[:], vbias_d[:, :])

        # initial hidden state, transposed to feature-major [P, 4, B]
        h0t = []
        for l in range(L):
            hsb = wpool.tile([B, D], F32, tag="h0load")
            nc.sync.dma_start(hsb[:], h0_d[l])
            ht = const.tile([P, 4 * B], BF16, tag=f"h0t{l}")
            for kk in range(4):
                tp = ps_tp.tile([P, P], F32, tag="tp")
                nc.tensor.transpose(tp[:, :B], hsb[:, kk * P:(kk + 1) * P],
                                    ident[:B, :B])
                nc.vector.tensor_copy(ht[:, kk * B:(kk + 1) * B], tp[:, :B])
            h0t.append(ht)

        # per-layer lhsT slice helpers
        def wx_lhsT(l, k, m):
            return wx_sb[l][:, k * GF + m * P: k * GF + (m + 1) * P]

        def wh_lhsT(l, k, m):
            return wh_sb[l][:, k * GF + m * P: k * GF + (m + 1) * P]

        # rolling stage state (python refs to previous chunk's tiles)
        prev_y = [None, None]     # previous chunk ystage per layer
        xst_cur = [None]          # current chunk xstage
        p_cur = [None, None]      # current chunk gate-preact stage per layer
        y_cur = [None, None]      # current chunk ystage per layer

        # ------------------------------------------------------------------
        def emit_gather(c):
            xst = xpool.tile([P, 4, TOKC], BF16, tag="xst")
            for g in range(GPC):
                gt = c * GPC + g
                it = ipool.tile([P, 1], I32, tag="idx")
                nc.sync.dma_start(it[:], idx_d[gt * P:(gt + 1) * P, :])
                xg = gpool.tile([P, D], F32, tag="xg")
                nc.gpsimd.indirect_dma_start(
                    out=xg[:], out_offset=None, in_=embed_d[:, :],
                    in_offset=bass.IndirectOffsetOnAxis(ap=it[:, :1], axis=0))
                for k in range(4):
                    tp = ps_tp.tile([P, P], F32, tag="tp")
                    nc.tensor.transpose(tp[:], xg[:, k * P:(k + 1) * P], ident[:])
                    nc.scalar.copy(xst[:, k, g * P:(g + 1) * P], tp[:])
            xst_cur[0] = xst

        # ------------------------------------------------------------------
        def emit_precompute(l, c):
            """x-projection GEMM for chunk c of layer l -> pstage (bf16)."""
            if l == 0:
                rhs = [xst_cur[0][:, k, :] for k in range(4)]
            else:
                rhs = [prev_or_cur_y(l - 1)[:, k, :] for k in range(4)]
            pst = ppool.tile([P, NM, TOKC], BF16, tag=f"pst{l}")
            for m in range(NM):
                ps = ps_big.tile([P, 512], F32, tag="big")
                for k in range(4):
                    nc.tensor.matmul(ps[:, :TOKC], wx_lhsT(l, k, m), rhs[k],
                                     start=(k == 0), stop=(k == 3))
                nc.scalar.activation(pst[:, m, :], ps[:, :TOKC], AF.Identity,
                                     bias=ball_sb[:, l * NM + m: l * NM + m + 1],
                                     scale=1.0)
            p_cur[l] = pst

        def prev_or_cur_y(l):
            return y_cur[l][:]

        # ------------------------------------------------------------------
        def emit_scan_chunk(l, c):
            """16 recurrent steps for layer l, chunk c."""
            yst = ypool.tile([P, 4, TOKC], BF16, tag=f"ys{l}")
            pst = p_cur[l]
            for tl in range(CS):
                t = c * CS + tl
                # h(t-1) access
                if t == 0:
                    hb = h0t[l][:].rearrange("p (k b) -> p k b", k=4)
                    pc = 0
                elif tl == 0:
                    hb = prev_y[l][:]
                    pc = (CS - 1) * B
                else:
                    hb = yst[:]
                    pc = (tl - 1) * B
                tcol = tl * B

                h1p = hb[:, 0:2, pc:pc + B]   # [P, 2, B] old h1
                h2p = hb[:, 2:4, pc:pc + B]   # [P, 2, B] old h2

                # ---- half 1: update h1 using h2p ----
                emit_half(l, pst, yst, tcol,
                          h_in=h2p, h_self=h1p,
                          m_zr=M_ZR1, m_g=M_G1, out_k=0)
                h1n = yst[:, 0:2, tcol:tcol + B]
                # ---- half 2: update h2 using h1n ----
                emit_half(l, pst, yst, tcol,
                          h_in=h1n, h_self=h2p,
                          m_zr=M_ZR2, m_g=M_G2, out_k=2)
            prev_y[l] = yst
            y_cur[l] = yst

        def emit_half(l, pst, yst, tcol, h_in, h_self, m_zr, m_g, out_k):
            """One reversible-GRU half-update.

            u_zr = P_zr + h_in @ Wzr_h.T ; s = sigmoid(u_zr); z*, r = s
            u_g = P_g + (r*h_in) @ Wg_h.T ; g = tanh(u_g)
            h_new = (0.875 z* + 0.125) h_self + 0.875 (1 - z*) g
            """
            ps_zr = ps_scan.tile([P, 192], F32, tag="sps")
            for m in range(4):
                for k in range(2):
                    nc.tensor.matmul(
                        ps_zr[:, m * B:(m + 1) * B],
                        wh_lhsT(l, k, m_zr + m), h_in[:, k, :],
                        start=(k == 0), stop=(k == 1))
            u_zr = wpool.tile([P, 4, B], F32, tag="uzr")
            nc.vector.scalar_tensor_tensor(
                out=u_zr[:], in0=ps_zr[:, 0:128].rearrange("p (m b) -> p m b", m=4),
                scalar=0.0, in1=pst[:, m_zr:m_zr + 4, tcol:tcol + B],
                op0=ALU.add, op1=ALU.add)
            sig = wpool.tile([P, 4, B], BF16, tag="sig")
            nc.scalar.activation(sig[:], u_zr[:], AF.Sigmoid)
            zs = sig[:, 0:2, :]               # [P, 2, B]
            rs = sig[:, 2:4, :]
            # r * h_in
            rh = wpool.tile([P, 2, B], BF16, tag="rh")
            nc.vector.tensor_tensor(out=rh[:], in0=rs, in1=h_in, op=ALU.mult)
            # g matmuls
            for m in range(2):
                for k in range(2):
                    nc.tensor.matmul(
                        ps_zr[:, 128 + m * B: 128 + (m + 1) * B],
                        wh_lhsT(l, k, m_g + m), rh[:, k, :],
                        start=(k == 0), stop=(k == 1))
            u_g = wpool.tile([P, 2, B], F32, tag="ug")
            nc.vector.scalar_tensor_tensor(
                out=u_g[:], in0=ps_zr[:, 128:192].rearrange("p (m b) -> p m b", m=2),
                scalar=0.0, in1=pst[:, m_g:m_g + 2, tcol:tcol + B],
                op0=ALU.add, op1=ALU.add)
            g = wpool.tile([P, 2, B], BF16, tag="g")
            nc.scalar.activation(g[:], u_g[:], AF.Tanh)
            # off-critical-path: zp = MF*z + (1-MF); w = MF*(1-z); a = zp*h_self
            zp = wpool.tile([P, 2, B], BF16, tag="zp")
            nc.vector.tensor_scalar(zp[:], zs, MF, 1.0 - MF, ALU.mult, ALU.add)
            w = wpool.tile([P, 2, B], BF16, tag="w")
            nc.vector.tensor_scalar(w[:], zs, -MF, MF, ALU.mult, ALU.add)
            a = wpool.tile([P, 2, B], BF16, tag="a")
            nc.vector.tensor_tensor(out=a[:], in0=zp[:], in1=h_self, op=ALU.mult)
            # critical: cgw = g*w ; h_new = cgw + a
            cgw = wpool.tile([P, 2, B], BF16, tag="cgw")
            nc.vector.tensor_tensor(out=cgw[:], in0=g[:], in1=w[:], op=ALU.mult)
            nc.vector.tensor_tensor(out=yst[:, out_k:out_k + 2, tcol:tcol + B],
                                    in0=cgw[:], in1=a[:], op=ALU.add)

        # ------------------------------------------------------------------
        def emit_decode(c):
            """Tied-decoder GEMM for the 4 token-tiles of chunk c."""
            yst = y_cur[1]
            for mi in range(GPC):
                tok0 = mi * P
                lhs = [yst[:, k, tok0:tok0 + P] for k in range(4)]
                mt = c * GPC + mi
                for n in range(9):
                    nsz = 512 if n < 8 else VS - 8 * 512
                    ps = ps_big.tile([P, 512], F32, tag="big")
                    for k in range(4):
                        nc.tensor.matmul(
                            ps[:, :nsz], lhs[k],
                            embt_sb[:, k * VS + n * 512: k * VS + n * 512 + nsz],
                            start=(k == 0), stop=(k == 3))
                    ot = opool.tile([P, 512], F32, tag="ot")
                    nc.vector.scalar_tensor_tensor(
                        out=ot[:, :nsz], in0=ps[:, :nsz], scalar=0.0,
                        in1=vbias_sb[:, n * 512: n * 512 + nsz],
                        op0=ALU.add, op1=ALU.add)
                    nc.sync.dma_start(
                        logits_d[mt * P:(mt + 1) * P, n * 512: n * 512 + nsz],
                        ot[:, :nsz])

        # ------------------------------------------------------------------
        def emit_hlast(l):
            yst = y_cur[l]
            hp = ps_tp.tile([B, 4 * P], BF16, tag="tp")
            for kk in range(4):
                nc.tensor.transpose(hp[:, kk * P:(kk + 1) * P],
                                    yst[:, kk, (CS - 1) * B: CS * B],
                                    identb[:])
            hs = wpool.tile([B, D], F32, tag="hlast")
            nc.vector.tensor_copy(hs[:], hp[:])
            nc.sync.dma_start(hlast_d[l], hs[:])

        # ------------------------------------------------------------------
        # pipeline emission
        for c in range(NCH):
            emit_gather(c)
            emit_precompute(0, c)
            emit_scan_chunk(0, c)
            emit_precompute(1, c)
            emit_scan_chunk(1, c)
            emit_decode(c)
        emit_hlast(0)
        emit_hlast(1)

    nc.compile()
    return nc


# ---------------------------------------------------------------------------
# Host side
# ---------------------------------------------------------------------------

_prog_cache = {}


def _get_program():
    if "nc" not in _prog_cache:
        _prog_cache["nc"] = build_program()
    return _prog_cache["nc"]


def _prep_inputs(input_seq, hiddens, embed, out_bias,
                 W_zr1, b_zr1, W_g1, b_g1, W_zr2, b_zr2, W_g2, b_g2):
    idx = np.asarray(input_seq).astype(np.int32).reshape(NT, 1)
    h0 = np.asarray(hiddens, dtype=np.float32)
    embed = np.ascontiguousarray(np.asarray(embed, dtype=np.float32))

    # gate weight packing: x-part lhsT [L, D, GF] (bf16), h-part lhsT [L, H2, GF]
    W_zr1 = np.asarray(W_zr1, np.float32)
    W_g1 = np.asarray(W_g1, np.float32)
    W_zr2 = np.asarray(W_zr2, np.float32)
    W_g2 = np.asarray(W_g2, np.float32)
    wx = np.concatenate([W_zr1[:, :, :D], W_g1[:, :, :D],
                         W_zr2[:, :, :D], W_g2[:, :, :D]], axis=1)  # [L, GF, D]
    wx = np.ascontiguousarray(wx.transpose(0, 2, 1)).astype(ml_dtypes.bfloat16)
    whp = np.concatenate([W_zr1[:, :, D:], W_g1[:, :, D:],
                          W_zr2[:, :, D:], W_g2[:, :, D:]], axis=1)  # [L, GF, H2]
    whp = np.ascontiguousarray(whp.transpose(0, 2, 1)).astype(ml_dtypes.bfloat16)
    ball = np.concatenate([np.asarray(b_zr1, np.float32),
                           np.asarray(b_g1, np.float32),
                           np.asarray(b_zr2, np.float32),
                           np.asarray(b_g2, np.float32)], axis=1)  # [L, GF]
    ball = np.ascontiguousarray(ball.reshape(L, NM, P))

    ob = np.zeros(VP, np.float32)
    ob[:NTOK] = np.asarray(out_bias, np.float32)

    embed_pad = np.zeros((VP, D), np.float32)
    embed_pad[:NTOK] = embed

    in_maps = []
    for c in range(NCORES):
        sl = slice(c * VS, (c + 1) * VS)
        embt = np.ascontiguousarray(embed_pad[sl].T).astype(ml_dtypes.bfloat16)
        vbias = np.ascontiguousarray(
            np.broadcast_to(ob[sl][None, :], (P, VS))).astype(ml_dtypes.bfloat16)
        in_maps.append({
            "idx": idx, "h0": h0, "embed": embed,
            "embt": embt, "vbias": vbias,
            "wx": wx, "wh": whp, "ball": ball,
        })
    return in_maps


def run(inputs, trace=False, tmpdir=None):
    nc = _get_program()
    in_maps = _prep_inputs(**inputs)
    res = run_bass_kernel_spmd(nc, in_maps, list(range(NCORES)), trace=trace,
                               tmpdir=tmpdir)
    logits = np.concatenate([res.results[c]["logits"] for c in range(NCORES)],
                            axis=1)[:, :NTOK]
    decoded = logits.reshape(S, B, NTOK)
    hlast = res.results[0]["hlast"]
    return (decoded, hlast), res


def kernel(**inputs):
    out, _ = run(inputs)
    return out
